# revision 17
# baseline (speedup 1.0000x reference)
"""Trainium2 Bass kernel for nn_BaseTransformer (ensemble member-attention).

Sharding: data-parallel over batch B=8 across 8 NeuronCores (1 batch each).

v4 design (vs v3 baseline at ~342us):
  - Custom DVE op ANT_FUSED_SELU16 computes the whole k/q SELU in ONE
    1x DVE pass from PSUM (exp via (1+z/16)^16 = 4 chained squarings,
    minimax-tuned constants; conv weights pre-scaled by s1 so the op's
    unit-coefficient affine matches; gram scale absorbs s1^2).
  - Custom DVE op ANT_SELU_COMBINE fuses the final selu's
    min(e - lam*a, relu(po + lam*b)) into one op (Act does only exp).
  - SBUF->SBUF DMA shuffles for the member<->group layout swaps (no
    DRAM scratch round-trip: HBM traffic 50.4MB -> 16.8MB per core).
  - Elementwise work split across DVE / Act / Pool(gpsimd) by static
    assignment tables; value conv interleaved into phase 1 to keep the
    PE dense (HAM warm); s-quartered pipelines bound SBUF usage.
"""

import sys

if "/opt/trn_rl_repo" not in sys.path:
    sys.path.insert(0, "/opt/trn_rl_repo")

import numpy as np

import concourse.bass as bass
import concourse.bacc as bacc
import concourse.mybir as mybir
import concourse.tile as tile



F32 = mybir.dt.float32
BF16 = mybir.dt.bfloat16

K, C, HEADS, S = 16, 64, 64, 4096
NG = 8
SC1 = 128           # phase-1 s-chunk (gram contraction tile)
NCH1 = S // SC1     # 32
SQ = 1024           # s-quarter size

ALPHA = 1.6732632423543772
LAMBDA = 1.0507009873554805
LN_LAMBDA_ALPHA = float(np.log(LAMBDA * ALPHA))
LAM_BF16 = 1.046875  # bf16(lambda); residual matmul uses this exactly

# fused-selu16 minimax constants (tuned offline: max err 0.0082 in selu/lam
# units over z in [-12, 5]); S1 is folded into the wk/wq conv weights.
S1 = 0.05176808915179282
SELU16_C0 = 0.8576971044920265
SELU16_C1 = 0.08619735530101351
GRAM_SCALE = float(LAMBDA * LAMBDA / (64.0 * S1 * S1))

# ---- engine split tables (tuned by trace) --------------------------------
# phase-1 selu chunks: 64 = 32 sc x {K,Q}.  True -> alt path (Act exp +
# Pool ts/stt), False -> fused custom op on DVE.
P1_ALT = [False for i in range(64)]
# value-conv copy engine per chunk (32 chunks of [128,1024])
VAL_COPY_ENG = (["act"] * 32)
# phase-2 combine path: True -> B-path (Act relu + pool sub + DVE tt)
P2_B = [False for i in range(32)]
# mix copy engine per (quarter, g) (32)
MIX_COPY_ENG = (["act", "act", "dve", "act", "act", "dve", "act", "act"] * 4)
# phase-2 combine: True -> pool 2-op path, False -> DVE SELU_COMBINE (32)
P2_POOL = [False for i in range(32)]  # unused


def _sigma(p):
    # storage head position p = 8g+u holds real head 8u+g
    return 8 * (p % 8) + (p // 8)


# ---------------- custom DVE op registration ------------------------------

def _register_op(name, spec):
    import concourse.dve_ops as dve_ops
    from concourse.dve_spec import lower, _has_src1
    from concourse.dve_uop import DveOpSpec

    if name in dve_ops._SUB_OPCODE_FOR_NAME:
        return next(op for op in dve_ops.OPS if op.name == name)
    row = dve_ops._CUSTOM_DVE_ROW_BASE + len(dve_ops.OPS)
    assert row < 0x20
    shas = {}
    for ver in ("v3", "v4"):
        ds = DveOpSpec(name=name, opcode=row, uops=lower(spec, ver=ver),
                       rd1_en=_has_src1(spec))
        shas[ver] = ds.sha(ver)
    op = dve_ops.DveOp(name, spec, subdim=False, uops_sha=shas)
    dve_ops.OPS.append(op)
    dve_ops._SUB_OPCODE_FOR_NAME[name] = row
    dve_ops.CUSTOM_DVE_SPECS[name] = spec
    for ver in ("v3", "v4"):
        op.compile(ver)
    return op


def _get_ops():
    from concourse.dve_spec import Spec, Src0, Src1, C0, C1, relu, sq, minn

    fused = _register_op(
        "ANT_FUSED_SELU16",
        Spec(
            body=minn(sq(sq(sq(sq(Src0 + C0)))) - C1, relu(Src0)),
            reference=lambda in0, s0, s1, imm2: np.minimum(
                (in0 + s0) ** 16 - s1, np.maximum(in0, 0)),
        ),
    )
    comb = _register_op(
        "ANT_SELU_COMBINE",
        Spec(
            body=minn(Src0 - C0, relu(Src1 + C1)),
            reference=lambda in0, in1, s0, s1, imm2: np.minimum(
                in0 - s0, np.maximum(in1 + s1, 0)),
        ),
    )
    return fused, comb


# ---------------- host-side constants -------------------------------------

def host_constants(w_value, w_key, w_query, w_out, b_out):
    consts = {}

    # combined k+q conv rhs [128,256]: col = h*128 + m2'*64 + p =
    # delta(m2,m2')*s1*W_h[p,c]  (position p = 8u+g holds head p; group
    # index g is the innermost col stride of kqT so gram operands are
    # single-free-dim strided APs)
    wkq2 = np.zeros((128, 256), np.float32)
    for h, w in ((0, w_key), (1, w_query)):
        for m2 in range(2):
            for p in range(64):
                wkq2[m2 * 64:(m2 + 1) * 64,
                     h * 128 + m2 * 64 + p] = S1 * w[p, :]
    consts["wkq2"] = wkq2

    # value conv lhsT [128,128]: [(m2,c),(a,p)] = delta(m2,a)*Wv[sigma(p),c]
    wv2 = np.zeros((128, 128), np.float32)
    for a in range(2):
        for p in range(64):
            wv2[a * 64:(a + 1) * 64, a * 64 + p] = w_value[_sigma(p), :]
    consts["wv2"] = wv2

    # out conv lhsT [128,128]: [(j2,p'),(a',o)] = delta(j2,a')*lam*Wout[o,p']
    # (tp rows use identity head order: row 64*j2 + p' holds head p')
    wo2 = np.zeros((128, 128), np.float32)
    for j2 in range(2):
        for p in range(64):
            wo2[j2 * 64 + p, j2 * 64:(j2 + 1) * 64] = LAMBDA * w_out[:, p]
    consts["wo2"] = wo2

    consts["resI"] = (LAM_BF16 * np.eye(128)).astype(np.float32)

    # gram mask replicated for all 8 groups: [128, 1024]
    mask = np.zeros((128, 128), np.float32)
    for p in range(128):
        for f in range(128):
            if p % 8 == f % 8:
                mask[p, f] = 1.0
    consts["maskg8"] = np.tile(mask, (1, 8)).copy()

    def _pi(u, m):
        return 64 * (m % 2) + 8 * u + (m // 2)

    # P[(8m+u), pi(u,m)] = 1
    P = np.zeros((128, 128), np.float32)
    for u in range(8):
        for m in range(16):
            P[8 * m + u, _pi(u, m)] = 1.0
    consts["permP"] = P
    consts["permPp"] = P.copy()

    # dpat[pi(u,i), pi(u,j)] = delta(i,j) - 1/16
    D = np.zeros((128, 128), np.float32)
    for u in range(8):
        for i in range(16):
            for j in range(16):
                D[_pi(u, i), _pi(u, j)] = (1.0 if i == j else 0.0) - 1.0 / 16.0
    consts["dpat"] = D

    bo2 = np.concatenate([b_out, b_out]).astype(np.float32)
    consts["be_col"] = (bo2 + LN_LAMBDA_ALPHA).reshape(128, 1)
    consts["bra_col"] = (LAMBDA * bo2).reshape(128, 1)
    return consts


def make_in_maps(in_tensor, consts):
    import ml_dtypes
    in_maps = []
    for b in range(8):
        xb = np.ascontiguousarray(
            in_tensor[b].reshape(8, 128, S)).astype(ml_dtypes.bfloat16)
        m = {"xb": xb}
        m.update(consts)
        in_maps.append(m)
    return in_maps


# ---------------- kernel build --------------------------------------------

def build_nc():
    FUSED_SELU16, SELU_COMBINE = _get_ops()

    nc = bacc.Bacc("TRN2", target_bir_lowering=False, debug=False)

    xb_d = nc.dram_tensor("xb", [8, 128, S], BF16, kind="ExternalInput")
    wkq2_d = nc.dram_tensor("wkq2", [128, 256], F32, kind="ExternalInput")
    wv2_d = nc.dram_tensor("wv2", [128, 128], F32, kind="ExternalInput")
    wo2_d = nc.dram_tensor("wo2", [128, 128], F32, kind="ExternalInput")
    resI_d = nc.dram_tensor("resI", [128, 128], F32, kind="ExternalInput")
    mask_d = nc.dram_tensor("maskg8", [128, 1024], F32, kind="ExternalInput")
    permP_d = nc.dram_tensor("permP", [128, 128], F32, kind="ExternalInput")
    permPp_d = nc.dram_tensor("permPp", [128, 128], F32, kind="ExternalInput")
    dpat_d = nc.dram_tensor("dpat", [128, 128], F32, kind="ExternalInput")
    be_d = nc.dram_tensor("be_col", [128, 1], F32, kind="ExternalInput")
    bra_d = nc.dram_tensor("bra_col", [128, 1], F32, kind="ExternalInput")
    out_d = nc.dram_tensor("out", [8, 128, S], BF16, kind="ExternalOutput")

    LNS1A = float(np.log(S1 * ALPHA))
    S1A = float(S1 * ALPHA)
    INV_S1 = float(1.0 / S1)
    LA = float(LAMBDA * ALPHA)
    INV_LAM = float(1.0 / LAMBDA)

    with tile.TileContext(nc) as tc:
        with (
            tc.tile_pool(name="persist", bufs=1) as persist,
            tc.tile_pool(name="xpool", bufs=1) as xpool,
            tc.tile_pool(name="vgpool", bufs=1) as vgpool,
            tc.tile_pool(name="vq", bufs=1) as vqp,
        ):
            # ---- constants ----
            def load_cast(dram, shape, tag, dtype=BF16):
                f = persist.tile(shape, F32, tag=tag + "f")
                nc.sync.dma_start(out=f, in_=dram[:, :])
                if dtype == F32:
                    return f
                b = persist.tile(shape, dtype, tag=tag)
                nc.gpsimd.tensor_copy(b, f)
                return b

            wkq_sb = load_cast(wkq2_d, [128, 256], "wkq")
            wv_sb = load_cast(wv2_d, [128, 128], "wv")
            wo_sb = load_cast(wo2_d, [128, 128], "wo")
            resI_sb = load_cast(resI_d, [128, 128], "resI")
            mask_sb = load_cast(mask_d, [128, 1024], "mask", F32)
            permP_sb = load_cast(permP_d, [128, 128], "permP", F32)
            permPp_sb = load_cast(permPp_d, [128, 128], "permPp", F32)
            dpat_sb = load_cast(dpat_d, [128, 128], "dpat", F32)
            be_sb = persist.tile([128, 1], F32, tag="be")
            nc.sync.dma_start(out=be_sb, in_=be_d[:, :])
            bra_sb = persist.tile([128, 1], F32, tag="bra")
            nc.sync.dma_start(out=bra_sb, in_=bra_d[:, :])
            lns1a_sb = persist.tile([128, 1], F32, tag="lns1a")
            nc.vector.memset(lns1a_sb, LNS1A)
            zero_sb = persist.tile([128, 1], F32, tag="zero")
            nc.vector.memset(zero_sb, 0.0)

            # ---- x tiles ----
            x_sb = []
            for t in range(8):
                xt = xpool.tile([128, S], BF16, tag=f"x{t}")
                nc.sync.dma_start(out=xt, in_=xb_d[t])
                x_sb.append(xt)

            # vg: 8 group tiles [128, 4096]
            vg_sb = [vgpool.tile([128, S], BF16, tag=f"vg{g}", name=f"vg{g}")
                     for g in range(NG)]

            bigB = [persist.tile([128, 128], BF16, tag=f"bigB{g}",
                                 name=f"bigB{g}") for g in range(NG)]

            # ================= phase 1 =================
            # psum: kq pool 3 x [128,1024] (6 banks), gram [128,1024] (2)
            with (
                tc.tile_pool(name="kqps", bufs=3, space="PSUM") as kqps,
                tc.tile_pool(name="gramps", bufs=1, space="PSUM") as gramps,
                tc.tile_pool(name="kqT", bufs=2) as kqTp,
                tc.tile_pool(name="stg", bufs=2) as stg,
            ):
                gram_ps = gramps.tile([128, 1024], F32, tag="gram")

                vquarter = [None]

                def value_chunk(ci, psum_pool, psum_tag):
                    # ci in 0..31; quarter q = ci//8, tile t = ci%8
                    q = ci // 8
                    t = ci % 8
                    if t == 0:
                        vqt = vqp.tile([128, 8 * SQ], BF16, tag="vq",
                                       name="vq")
                        vquarter[0] = vqt
                    vh = vquarter[0]
                    vp = psum_pool.tile([128, 1024], F32, tag=psum_tag,
                                        name="vp")
                    s0 = SQ * q
                    nc.tensor.matmul(vp[:, 0:512], wv_sb,
                                     x_sb[t][:, s0: s0 + 512],
                                     start=True, stop=True)
                    nc.tensor.matmul(vp[:, 512:1024], wv_sb,
                                     x_sb[t][:, s0 + 512: s0 + 1024],
                                     start=True, stop=True)
                    dst = vh[:, t * SQ: (t + 1) * SQ]
                    eng = VAL_COPY_ENG[ci]
                    if eng == "act":
                        nc.scalar.copy(dst, vp)
                    else:
                        nc.vector.tensor_copy(dst, vp)

                def vshuffle_quarter(q):
                    # v-quarter [128, (t:8, s:1024)] -> vg[g][:, 1024q:+1024]
                    vh = vquarter[0]
                    for g in range(NG):
                        for a in range(2):
                            src = vh[64 * a + 8 * g: 64 * a + 8 * g + 8,
                                     :].rearrange("u (b s) -> u b s", b=8)
                            nc.gpsimd.dma_start(
                                out=vg_sb[g][64 * a: 64 * (a + 1),
                                             SQ * q: SQ * (q + 1)],
                                in_=src)

                def selu_chunk(ps, dst, idx):
                    if not P1_ALT[idx]:
                        nc.vector._custom_dve(
                            FUSED_SELU16, out=dst, in0=ps,
                            s0=SELU16_C0, s1=SELU16_C1)
                    else:
                        e = stg.tile([128, 1024], BF16, tag="e")
                        nc.scalar.activation(
                            out=e, in_=ps,
                            func=mybir.ActivationFunctionType.Exp,
                            bias=lns1a_sb[:, 0:1], scale=INV_S1)
                        r = stg.tile([128, 1024], BF16, tag="r")
                        nc.scalar.activation(
                            out=r, in_=ps,
                            func=mybir.ActivationFunctionType.Relu,
                            bias=zero_sb[:, 0:1])
                        ep = stg.tile([128, 1024], BF16, tag="ep")
                        nc.gpsimd.tensor_scalar(
                            out=ep, in0=e, scalar1=S1A, scalar2=None,
                            op0=mybir.AluOpType.subtract)
                        nc.vector.tensor_tensor(
                            out=dst, in0=ep, in1=r,
                            op=mybir.AluOpType.min)

                for sc in range(NCH1):
                    sl = slice(SC1 * sc, SC1 * (sc + 1))
                    kqT = kqTp.tile([128, 2048], BF16, tag="kqT")
                    psA = kqps.tile([128, 1024], F32, tag="kq")
                    psB = kqps.tile([128, 1024], F32, tag="kq")
                    for ti in range(4):
                        nc.tensor.matmul(psA[:, 256 * ti: 256 * (ti + 1)],
                                         x_sb[ti][:, sl], wkq_sb,
                                         start=True, stop=True)
                        nc.tensor.matmul(psB[:, 256 * ti: 256 * (ti + 1)],
                                         x_sb[4 + ti][:, sl], wkq_sb,
                                         start=True, stop=True)
                    # psX col = 256*ti + h*128 + (m2*64+p); kqT col =
                    # h*1024 + 128*t + (m2*64+p)
                    for half, ps in ((0, psA), (1, psB)):
                        pv = ps.rearrange("s (ti z) -> s ti z", ti=4, z=256)
                        for h in range(2):
                            dst = kqT[:, 1024 * h + 512 * half:
                                      1024 * h + 512 * (half + 1)].rearrange(
                                "s (ti z) -> s ti z", ti=4, z=128)
                            nc.vector._custom_dve(
                                FUSED_SELU16, out=dst,
                                in0=pv[:, :, 128 * h: 128 * (h + 1)],
                                s0=SELU16_C0, s1=SELU16_C1)

                    # gram: lhsT = q-AP, rhs = k-AP (partition j, free i);
                    # kqT col = h*1024 + f*8 + g with f = 8m+u, head = 8u+g
                    kv = kqT.rearrange("s (h f g) -> s h g f",
                                       h=2, f=128, g=8)
                    for g in range(NG):
                        nc.tensor.matmul(
                            gram_ps[:, 128 * g: 128 * (g + 1)],
                            kv[:, 1, g], kv[:, 0, g],
                            start=(sc == 0), stop=(sc == NCH1 - 1))

                    # interleave value conv (1 chunk per sc)
                    value_chunk(sc, kqps, "kq")
                    if sc % 8 == 7:
                        vshuffle_quarter(sc // 8)

                # ---- softmax + bigB ----
                E = stg.tile([128, 1024], F32, tag="E")
                nc.scalar.activation(
                    out=E, in_=gram_ps,
                    func=mybir.ActivationFunctionType.Exp,
                    bias=zero_sb[:, 0:1], scale=GRAM_SCALE)
                nc.vector.tensor_tensor(out=E, in0=E, in1=mask_sb,
                                        op=mybir.AluOpType.mult)
                Ssum = stg.tile([128, 8], F32, tag="Ssum")
                nc.vector.tensor_reduce(
                    out=Ssum,
                    in_=E.rearrange("p (g f) -> p g f", g=8),
                    axis=mybir.AxisListType.X, op=mybir.AluOpType.add)
                R = stg.tile([128, 8], F32, tag="R")
                nc.vector.reciprocal(out=R, in_=Ssum)
                for g in range(NG):
                    nc.vector.tensor_scalar(
                        out=E[:, 128 * g: 128 * (g + 1)],
                        in0=E[:, 128 * g: 128 * (g + 1)],
                        scalar1=R[:, g: g + 1], scalar2=None,
                        op0=mybir.AluOpType.mult)
                for g in range(NG):
                    c_ps = kqps.tile([128, 1024], F32, tag="kq")
                    nc.tensor.matmul(c_ps[:, 0:128],
                                     E[:, 128 * g: 128 * (g + 1)],
                                     permPp_sb, start=True, stop=True)
                    c_sb = stg.tile([128, 128], F32, tag="csb")
                    nc.scalar.copy(c_sb, c_ps[:, 0:128])
                    b_ps = kqps.tile([128, 1024], F32, tag="kq")
                    nc.tensor.matmul(b_ps[:, 0:128], permP_sb, c_sb,
                                     start=True, stop=True)
                    nc.vector.scalar_tensor_tensor(
                        out=bigB[g], in0=b_ps[:, 0:128], scalar=1.0,
                        in1=dpat_sb,
                        op0=mybir.AluOpType.mult, op1=mybir.AluOpType.add)

            # ================= phase 2 =================
            with (
                tc.tile_pool(name="mixps", bufs=2, space="PSUM") as mixps,
                tc.tile_pool(name="outps", bufs=2, space="PSUM") as outps,
                tc.tile_pool(name="tgp", bufs=1) as tgp,
                tc.tile_pool(name="tpp", bufs=1) as tpp,
                tc.tile_pool(name="stg2", bufs=2) as stg2,
                tc.tile_pool(name="outst", bufs=2) as outst,
            ):
                comb_idx = 0
                mix_idx = 0
                for q in range(4):
                    s0q = SQ * q
                    # mix: tg [128, (g:8, s:1024)]
                    tg = tgp.tile([128, 8 * SQ], BF16, tag="tg")
                    for g in range(NG):
                        pm = mixps.tile([128, 1024], F32, tag="m")
                        nc.tensor.matmul(
                            pm[:, 0:512], bigB[g],
                            vg_sb[g][:, s0q: s0q + 512],
                            start=True, stop=True)
                        nc.tensor.matmul(
                            pm[:, 512:1024], bigB[g],
                            vg_sb[g][:, s0q + 512: s0q + 1024],
                            start=True, stop=True)
                        dst = tg[:, SQ * g: SQ * (g + 1)]
                        eng = MIX_COPY_ENG[mix_idx]
                        mix_idx += 1
                        if eng == "act":
                            nc.scalar.copy(dst, pm)
                        else:
                            nc.vector.tensor_copy(dst, pm)

                    # t-shuffle: tg -> tp tiles
                    tg_v = tg.rearrange("(j2 u t) (g s) -> j2 t u g s",
                                        j2=2, u=8, t=8, g=8)
                    tp_t = []
                    for t in range(8):
                        tpt = tpp.tile([128, SQ], BF16, tag=f"tp{t}")
                        for j2 in range(2):
                            nc.gpsimd.dma_start(
                                out=tpt[64 * j2: 64 * (j2 + 1), :],
                                in_=tg_v[j2, t])
                        tp_t.append(tpt)

                    # out conv + final selu
                    for t in range(8):
                        po = outps.tile([128, 1024], F32, tag="o")
                        nc.tensor.matmul(
                            po[:, 0:512], wo_sb, tp_t[t][:, 0:512],
                            start=True, stop=False)
                        nc.tensor.matmul(
                            po[:, 512:1024], wo_sb, tp_t[t][:, 512:1024],
                            start=True, stop=False)
                        nc.tensor.matmul(
                            po[:, 0:512], resI_sb,
                            x_sb[t][:, s0q: s0q + 512],
                            start=False, stop=True)
                        nc.tensor.matmul(
                            po[:, 512:1024], resI_sb,
                            x_sb[t][:, s0q + 512: s0q + 1024],
                            start=False, stop=True)
                        e2 = stg2.tile([128, 1024], F32, tag="e2")
                        nc.scalar.activation(
                            out=e2, in_=po,
                            func=mybir.ActivationFunctionType.Exp,
                            bias=be_sb[:, 0:1], scale=INV_LAM)
                        ot = outst.tile([128, 1024], BF16, tag="ot")
                        if not P2_B[comb_idx]:
                            nc.vector._custom_dve(
                                SELU_COMBINE, out=ot, in0=e2, in1=po,
                                s0=LA, s1=bra_sb[:, 0:1])
                        else:
                            rf = stg2.tile([128, 1024], BF16, tag="rf")
                            nc.scalar.activation(
                                out=rf, in_=po,
                                func=mybir.ActivationFunctionType.Relu,
                                bias=bra_sb[:, 0:1])
                            e2p = stg2.tile([128, 1024], BF16, tag="e2p")
                            nc.gpsimd.tensor_scalar(
                                out=e2p, in0=e2, scalar1=LA, scalar2=None,
                                op0=mybir.AluOpType.subtract)
                            nc.vector.tensor_tensor(
                                out=ot, in0=e2p, in1=rf,
                                op=mybir.AluOpType.min)
                        comb_idx += 1
                        nc.sync.dma_start(
                            out=out_d[t][:, s0q: s0q + 1024], in_=ot)
    nc.compile()
    return nc


_NC_CACHE = None


def _get_nc():
    global _NC_CACHE
    if _NC_CACHE is None:
        _NC_CACHE = build_nc()
    return _NC_CACHE


def kernel(in_tensor, w_value, w_key, w_query, w_out, b_out, **_ignored):
    in_tensor = np.asarray(in_tensor, dtype=np.float32)
    consts = host_constants(
        np.asarray(w_value, dtype=np.float32),
        np.asarray(w_key, dtype=np.float32),
        np.asarray(w_query, dtype=np.float32),
        np.asarray(w_out, dtype=np.float32),
        np.asarray(b_out, dtype=np.float32))
    assert in_tensor.shape[0] == 8
    in_maps = make_in_maps(in_tensor, consts)

    nc = _get_nc()
    from concourse.bass_utils import run_bass_kernel_spmd
    res = run_bass_kernel_spmd(nc, in_maps, core_ids=list(range(8)))
    outs = [np.asarray(res.results[b]["out"]).astype(np.float32)
            .reshape(K, C, 64, 64) for b in range(8)]
    return np.stack(outs, axis=0)


if __name__ == "__main__":
    build_nc()
    print("built ok")


# revision 19
# speedup vs baseline: 1.0540x; 1.0540x over previous
"""Trainium2 Bass kernel for nn_BaseTransformer (ensemble member-attention).

Sharding: data-parallel over batch B=8 across 8 NeuronCores (1 batch each).

v4 design (vs v3 baseline at ~342us):
  - Custom DVE op ANT_FUSED_SELU16 computes the whole k/q SELU in ONE
    1x DVE pass from PSUM (exp via (1+z/16)^16 = 4 chained squarings,
    minimax-tuned constants; conv weights pre-scaled by s1 so the op's
    unit-coefficient affine matches; gram scale absorbs s1^2).
  - Custom DVE op ANT_SELU_COMBINE fuses the final selu's
    min(e - lam*a, relu(po + lam*b)) into one op (Act does only exp).
  - SBUF->SBUF DMA shuffles for the member<->group layout swaps (no
    DRAM scratch round-trip: HBM traffic 50.4MB -> 16.8MB per core).
  - Elementwise work split across DVE / Act / Pool(gpsimd) by static
    assignment tables; value conv interleaved into phase 1 to keep the
    PE dense (HAM warm); s-quartered pipelines bound SBUF usage.
"""

import sys

if "/opt/trn_rl_repo" not in sys.path:
    sys.path.insert(0, "/opt/trn_rl_repo")

import numpy as np

import concourse.bass as bass
import concourse.bacc as bacc
import concourse.mybir as mybir
import concourse.tile as tile



F32 = mybir.dt.float32
BF16 = mybir.dt.bfloat16

K, C, HEADS, S = 16, 64, 64, 4096
NG = 8
SC1 = 128           # phase-1 s-chunk (gram contraction tile)
NCH1 = S // SC1     # 32
SQ = 1024           # s-quarter size

ALPHA = 1.6732632423543772
LAMBDA = 1.0507009873554805
LN_LAMBDA_ALPHA = float(np.log(LAMBDA * ALPHA))
LAM_BF16 = 1.046875  # bf16(lambda); residual matmul uses this exactly

# fused-selu16 minimax constants (tuned offline: max err 0.0082 in selu/lam
# units over z in [-12, 5]); S1 is folded into the wk/wq conv weights.
S1 = 0.05176808915179282
SELU16_C0 = 0.8576971044920265
SELU16_C1 = 0.08619735530101351
GRAM_SCALE = float(LAMBDA * LAMBDA / (64.0 * S1 * S1))

# ---- engine split tables (tuned by trace) --------------------------------
# phase-1 selu chunks: 64 = 32 sc x {K,Q}.  True -> alt path (Act exp +
# Pool ts/stt), False -> fused custom op on DVE.
P1_ALT = [False for i in range(64)]
# value-conv copy engine per chunk (32 chunks of [128,1024])
VAL_COPY_ENG = (["act"] * 32)
# phase-2 combine path: True -> B-path (Act relu + pool sub + DVE tt)
P2_B = [False for i in range(32)]
# mix copy engine per (quarter, g) (32)
MIX_COPY_ENG = (["act", "act", "dve", "act", "act", "dve", "act", "act"] * 4)
# phase-2 combine: True -> pool 2-op path, False -> DVE SELU_COMBINE (32)
P2_POOL = [False for i in range(32)]  # unused


def _sigma(p):
    # storage head position p = 8g+u holds real head 8u+g
    return 8 * (p % 8) + (p // 8)


# ---------------- custom DVE op registration ------------------------------

def _register_op(name, spec):
    import concourse.dve_ops as dve_ops
    from concourse.dve_spec import lower, _has_src1
    from concourse.dve_uop import DveOpSpec

    if name in dve_ops._SUB_OPCODE_FOR_NAME:
        return next(op for op in dve_ops.OPS if op.name == name)
    row = dve_ops._CUSTOM_DVE_ROW_BASE + len(dve_ops.OPS)
    assert row < 0x20
    shas = {}
    for ver in ("v3", "v4"):
        ds = DveOpSpec(name=name, opcode=row, uops=lower(spec, ver=ver),
                       rd1_en=_has_src1(spec))
        shas[ver] = ds.sha(ver)
    op = dve_ops.DveOp(name, spec, subdim=False, uops_sha=shas)
    dve_ops.OPS.append(op)
    dve_ops._SUB_OPCODE_FOR_NAME[name] = row
    dve_ops.CUSTOM_DVE_SPECS[name] = spec
    for ver in ("v3", "v4"):
        op.compile(ver)
    return op


def _get_ops():
    from concourse.dve_spec import Spec, Src0, Src1, C0, C1, relu, sq, minn

    fused = _register_op(
        "ANT_FUSED_SELU16",
        Spec(
            body=minn(sq(sq(sq(sq(Src0 + C0)))) - C1, relu(Src0)),
            reference=lambda in0, s0, s1, imm2: np.minimum(
                (in0 + s0) ** 16 - s1, np.maximum(in0, 0)),
        ),
    )
    comb = _register_op(
        "ANT_SELU_COMBINE",
        Spec(
            body=minn(Src0 - C0, relu(Src1 + C1)),
            reference=lambda in0, in1, s0, s1, imm2: np.minimum(
                in0 - s0, np.maximum(in1 + s1, 0)),
        ),
    )
    return fused, comb


# ---------------- host-side constants -------------------------------------

def host_constants(w_value, w_key, w_query, w_out, b_out):
    consts = {}

    # combined k+q conv rhs [128,256]: col = h*128 + m2'*64 + p =
    # delta(m2,m2')*s1*W_h[p,c]  (position p = 8u+g holds head p; group
    # index g is the innermost col stride of kqT so gram operands are
    # single-free-dim strided APs)
    wkq2 = np.zeros((128, 256), np.float32)
    for h, w in ((0, w_key), (1, w_query)):
        for m2 in range(2):
            for p in range(64):
                wkq2[m2 * 64:(m2 + 1) * 64,
                     h * 128 + m2 * 64 + p] = S1 * w[p, :]
    consts["wkq2"] = wkq2

    # value conv lhsT [128,128]: [(m2,c),(a,p)] = delta(m2,a)*Wv[sigma(p),c]
    wv2 = np.zeros((128, 128), np.float32)
    for a in range(2):
        for p in range(64):
            wv2[a * 64:(a + 1) * 64, a * 64 + p] = w_value[_sigma(p), :]
    consts["wv2"] = wv2

    # out conv lhsT [128,128]: [(j2,p'),(a',o)] = delta(j2,a')*lam*Wout[o,p']
    # (tp rows use identity head order: row 64*j2 + p' holds head p')
    wo2 = np.zeros((128, 128), np.float32)
    for j2 in range(2):
        for p in range(64):
            wo2[j2 * 64 + p, j2 * 64:(j2 + 1) * 64] = LAMBDA * w_out[:, p]
    consts["wo2"] = wo2

    consts["resI"] = (LAM_BF16 * np.eye(128)).astype(np.float32)

    # gram mask replicated for all 8 groups: [128, 1024]
    mask = np.zeros((128, 128), np.float32)
    for p in range(128):
        for f in range(128):
            if p % 8 == f % 8:
                mask[p, f] = 1.0
    consts["maskg8"] = np.tile(mask, (1, 8)).copy()

    def _pi(u, m):
        return 64 * (m % 2) + 8 * u + (m // 2)

    # P[(8m+u), pi(u,m)] = 1
    P = np.zeros((128, 128), np.float32)
    for u in range(8):
        for m in range(16):
            P[8 * m + u, _pi(u, m)] = 1.0
    consts["permP"] = P
    consts["permPp"] = P.copy()

    # dpat[pi(u,i), pi(u,j)] = delta(i,j) - 1/16
    D = np.zeros((128, 128), np.float32)
    for u in range(8):
        for i in range(16):
            for j in range(16):
                D[_pi(u, i), _pi(u, j)] = (1.0 if i == j else 0.0) - 1.0 / 16.0
    consts["dpat"] = D

    bo2 = np.concatenate([b_out, b_out]).astype(np.float32)
    consts["be_col"] = (bo2 + LN_LAMBDA_ALPHA).reshape(128, 1)
    consts["bra_col"] = (LAMBDA * bo2).reshape(128, 1)
    return consts


def make_in_maps(in_tensor, consts):
    import ml_dtypes
    in_maps = []
    for b in range(8):
        xb = np.ascontiguousarray(
            in_tensor[b].reshape(8, 128, S)).astype(ml_dtypes.bfloat16)
        m = {"xb": xb}
        m.update(consts)
        in_maps.append(m)
    return in_maps


# ---------------- kernel build --------------------------------------------

def build_nc():
    FUSED_SELU16, SELU_COMBINE = _get_ops()

    nc = bacc.Bacc("TRN2", target_bir_lowering=False, debug=False)

    xb_d = nc.dram_tensor("xb", [8, 128, S], BF16, kind="ExternalInput")
    wkq2_d = nc.dram_tensor("wkq2", [128, 256], F32, kind="ExternalInput")
    wv2_d = nc.dram_tensor("wv2", [128, 128], F32, kind="ExternalInput")
    wo2_d = nc.dram_tensor("wo2", [128, 128], F32, kind="ExternalInput")
    resI_d = nc.dram_tensor("resI", [128, 128], F32, kind="ExternalInput")
    mask_d = nc.dram_tensor("maskg8", [128, 1024], F32, kind="ExternalInput")
    permP_d = nc.dram_tensor("permP", [128, 128], F32, kind="ExternalInput")
    permPp_d = nc.dram_tensor("permPp", [128, 128], F32, kind="ExternalInput")
    dpat_d = nc.dram_tensor("dpat", [128, 128], F32, kind="ExternalInput")
    be_d = nc.dram_tensor("be_col", [128, 1], F32, kind="ExternalInput")
    bra_d = nc.dram_tensor("bra_col", [128, 1], F32, kind="ExternalInput")
    out_d = nc.dram_tensor("out", [8, 128, S], BF16, kind="ExternalOutput")

    LNS1A = float(np.log(S1 * ALPHA))
    S1A = float(S1 * ALPHA)
    INV_S1 = float(1.0 / S1)
    LA = float(LAMBDA * ALPHA)
    INV_LAM = float(1.0 / LAMBDA)

    with tile.TileContext(nc) as tc:
        with (
            tc.tile_pool(name="persist", bufs=1) as persist,
            tc.tile_pool(name="xpool", bufs=1) as xpool,
            tc.tile_pool(name="vgpool", bufs=1) as vgpool,
        ):
            # ---- constants ----
            def load_cast(dram, shape, tag, dtype=BF16):
                f = persist.tile(shape, F32, tag=tag + "f")
                nc.sync.dma_start(out=f, in_=dram[:, :])
                if dtype == F32:
                    return f
                b = persist.tile(shape, dtype, tag=tag)
                nc.gpsimd.tensor_copy(b, f)
                return b

            wkq_sb = load_cast(wkq2_d, [128, 256], "wkq")
            wv_sb = load_cast(wv2_d, [128, 128], "wv")
            wo_sb = load_cast(wo2_d, [128, 128], "wo")
            resI_sb = load_cast(resI_d, [128, 128], "resI")
            mask_sb = load_cast(mask_d, [128, 1024], "mask", F32)
            permP_sb = load_cast(permP_d, [128, 128], "permP", F32)
            permPp_sb = load_cast(permPp_d, [128, 128], "permPp", F32)
            dpat_sb = load_cast(dpat_d, [128, 128], "dpat", F32)
            be_sb = persist.tile([128, 1], F32, tag="be")
            nc.sync.dma_start(out=be_sb, in_=be_d[:, :])
            bra_sb = persist.tile([128, 1], F32, tag="bra")
            nc.sync.dma_start(out=bra_sb, in_=bra_d[:, :])
            lns1a_sb = persist.tile([128, 1], F32, tag="lns1a")
            nc.vector.memset(lns1a_sb, LNS1A)
            zero_sb = persist.tile([128, 1], F32, tag="zero")
            nc.vector.memset(zero_sb, 0.0)

            # ---- x tiles ----
            x_sb = []
            for t in range(8):
                xt = xpool.tile([128, S], BF16, tag=f"x{t}")
                nc.sync.dma_start(out=xt, in_=xb_d[t])
                x_sb.append(xt)

            # vg: 8 group tiles [128, 4096]
            vg_sb = [vgpool.tile([128, S], BF16, tag=f"vg{g}", name=f"vg{g}")
                     for g in range(NG)]

            bigB = [persist.tile([128, 128], BF16, tag=f"bigB{g}",
                                 name=f"bigB{g}") for g in range(NG)]

            # ================= phase 1 =================
            # psum: kq pool 3 x [128,1024] (6 banks), gram [128,1024] (2)
            with (
                tc.tile_pool(name="kqps", bufs=3, space="PSUM") as kqps,
                tc.tile_pool(name="gramps", bufs=1, space="PSUM") as gramps,
                tc.tile_pool(name="kqT", bufs=2) as kqTp,
                tc.tile_pool(name="stg", bufs=2) as stg,
                tc.tile_pool(name="vq", bufs=2) as vqp,
            ):
                gram_ps = gramps.tile([128, 1024], F32, tag="gram")

                vquarter = [None]

                def value_chunk(ci, psum_pool, psum_tag):
                    # ci in 0..31; quarter q = ci//8, tile t = ci%8
                    q = ci // 8
                    t = ci % 8
                    if t == 0:
                        vqt = vqp.tile([128, 8 * SQ], BF16, tag="vq",
                                       name="vq")
                        vquarter[0] = vqt
                    vh = vquarter[0]
                    vp = psum_pool.tile([128, 1024], F32, tag=psum_tag,
                                        name="vp")
                    s0 = SQ * q
                    nc.tensor.matmul(vp[:, 0:512], wv_sb,
                                     x_sb[t][:, s0: s0 + 512],
                                     start=True, stop=True)
                    nc.tensor.matmul(vp[:, 512:1024], wv_sb,
                                     x_sb[t][:, s0 + 512: s0 + 1024],
                                     start=True, stop=True)
                    dst = vh[:, t * SQ: (t + 1) * SQ]
                    eng = VAL_COPY_ENG[ci]
                    if eng == "act":
                        nc.scalar.copy(dst, vp)
                    else:
                        nc.vector.tensor_copy(dst, vp)

                def vshuffle_quarter(q):
                    # v-quarter [128, (t:8, s:1024)] -> vg[g][:, 1024q:+1024]
                    vh = vquarter[0]
                    for g in range(NG):
                        for a in range(2):
                            src = vh[64 * a + 8 * g: 64 * a + 8 * g + 8,
                                     :].rearrange("u (b s) -> u b s", b=8)
                            nc.gpsimd.dma_start(
                                out=vg_sb[g][64 * a: 64 * (a + 1),
                                             SQ * q: SQ * (q + 1)],
                                in_=src)

                def selu_chunk(ps, dst, idx):
                    if not P1_ALT[idx]:
                        nc.vector._custom_dve(
                            FUSED_SELU16, out=dst, in0=ps,
                            s0=SELU16_C0, s1=SELU16_C1)
                    else:
                        e = stg.tile([128, 1024], BF16, tag="e")
                        nc.scalar.activation(
                            out=e, in_=ps,
                            func=mybir.ActivationFunctionType.Exp,
                            bias=lns1a_sb[:, 0:1], scale=INV_S1)
                        r = stg.tile([128, 1024], BF16, tag="r")
                        nc.scalar.activation(
                            out=r, in_=ps,
                            func=mybir.ActivationFunctionType.Relu,
                            bias=zero_sb[:, 0:1])
                        ep = stg.tile([128, 1024], BF16, tag="ep")
                        nc.gpsimd.tensor_scalar(
                            out=ep, in0=e, scalar1=S1A, scalar2=None,
                            op0=mybir.AluOpType.subtract)
                        nc.vector.tensor_tensor(
                            out=dst, in0=ep, in1=r,
                            op=mybir.AluOpType.min)

                for sc in range(NCH1):
                    sl = slice(SC1 * sc, SC1 * (sc + 1))
                    kqT = kqTp.tile([128, 2048], BF16, tag="kqT")
                    psA = kqps.tile([128, 1024], F32, tag="kq")
                    psB = kqps.tile([128, 1024], F32, tag="kq")
                    for ti in range(4):
                        nc.tensor.matmul(psA[:, 256 * ti: 256 * (ti + 1)],
                                         x_sb[ti][:, sl], wkq_sb,
                                         start=True, stop=True)
                        nc.tensor.matmul(psB[:, 256 * ti: 256 * (ti + 1)],
                                         x_sb[4 + ti][:, sl], wkq_sb,
                                         start=True, stop=True)
                    # psX col = 256*ti + h*128 + (m2*64+p); kqT col =
                    # h*1024 + 128*t + (m2*64+p)
                    for half, ps in ((0, psA), (1, psB)):
                        pv = ps.rearrange("s (ti z) -> s ti z", ti=4, z=256)
                        for h in range(2):
                            dst = kqT[:, 1024 * h + 512 * half:
                                      1024 * h + 512 * (half + 1)].rearrange(
                                "s (ti z) -> s ti z", ti=4, z=128)
                            nc.vector._custom_dve(
                                FUSED_SELU16, out=dst,
                                in0=pv[:, :, 128 * h: 128 * (h + 1)],
                                s0=SELU16_C0, s1=SELU16_C1)

                    # gram: lhsT = q-AP, rhs = k-AP (partition j, free i);
                    # kqT col = h*1024 + f*8 + g with f = 8m+u, head = 8u+g
                    kv = kqT.rearrange("s (h f g) -> s h g f",
                                       h=2, f=128, g=8)
                    for g in range(NG):
                        nc.tensor.matmul(
                            gram_ps[:, 128 * g: 128 * (g + 1)],
                            kv[:, 1, g], kv[:, 0, g],
                            start=(sc == 0), stop=(sc == NCH1 - 1))

                    # interleave value conv (1 chunk per sc)
                    value_chunk(sc, kqps, "kq")
                    if sc % 8 == 7:
                        vshuffle_quarter(sc // 8)

                # ---- softmax + bigB ----
                E = stg.tile([128, 1024], F32, tag="E")
                nc.scalar.activation(
                    out=E, in_=gram_ps,
                    func=mybir.ActivationFunctionType.Exp,
                    bias=zero_sb[:, 0:1], scale=GRAM_SCALE)
                nc.vector.tensor_tensor(out=E, in0=E, in1=mask_sb,
                                        op=mybir.AluOpType.mult)
                Ssum = stg.tile([128, 8], F32, tag="Ssum")
                nc.vector.tensor_reduce(
                    out=Ssum,
                    in_=E.rearrange("p (g f) -> p g f", g=8),
                    axis=mybir.AxisListType.X, op=mybir.AluOpType.add)
                R = stg.tile([128, 8], F32, tag="R")
                nc.vector.reciprocal(out=R, in_=Ssum)
                for g in range(NG):
                    nc.vector.tensor_scalar(
                        out=E[:, 128 * g: 128 * (g + 1)],
                        in0=E[:, 128 * g: 128 * (g + 1)],
                        scalar1=R[:, g: g + 1], scalar2=None,
                        op0=mybir.AluOpType.mult)
                for g in range(NG):
                    c_ps = kqps.tile([128, 1024], F32, tag="kq")
                    nc.tensor.matmul(c_ps[:, 0:128],
                                     E[:, 128 * g: 128 * (g + 1)],
                                     permPp_sb, start=True, stop=True)
                    c_sb = stg.tile([128, 128], F32, tag="csb")
                    nc.scalar.copy(c_sb, c_ps[:, 0:128])
                    b_ps = kqps.tile([128, 1024], F32, tag="kq")
                    nc.tensor.matmul(b_ps[:, 0:128], permP_sb, c_sb,
                                     start=True, stop=True)
                    nc.vector.scalar_tensor_tensor(
                        out=bigB[g], in0=b_ps[:, 0:128], scalar=1.0,
                        in1=dpat_sb,
                        op0=mybir.AluOpType.mult, op1=mybir.AluOpType.add)

            # ================= phase 2 =================
            with (
                tc.tile_pool(name="mixps", bufs=2, space="PSUM") as mixps,
                tc.tile_pool(name="outps", bufs=2, space="PSUM") as outps,
                tc.tile_pool(name="tgp", bufs=1) as tgp,
                tc.tile_pool(name="tpp", bufs=2) as tpp,
                tc.tile_pool(name="stg2", bufs=2) as stg2,
                tc.tile_pool(name="outst", bufs=2) as outst,
            ):
                comb_idx = 0
                mix_idx = 0
                for q in range(4):
                    s0q = SQ * q
                    # mix: tg [128, (g:8, s:1024)]
                    tg = tgp.tile([128, 8 * SQ], BF16, tag="tg")
                    for g in range(NG):
                        pm = mixps.tile([128, 1024], F32, tag="m")
                        nc.tensor.matmul(
                            pm[:, 0:512], bigB[g],
                            vg_sb[g][:, s0q: s0q + 512],
                            start=True, stop=True)
                        nc.tensor.matmul(
                            pm[:, 512:1024], bigB[g],
                            vg_sb[g][:, s0q + 512: s0q + 1024],
                            start=True, stop=True)
                        dst = tg[:, SQ * g: SQ * (g + 1)]
                        eng = MIX_COPY_ENG[mix_idx]
                        mix_idx += 1
                        if eng == "act":
                            nc.scalar.copy(dst, pm)
                        else:
                            nc.vector.tensor_copy(dst, pm)

                    # t-shuffle: tg -> tp tiles
                    tg_v = tg.rearrange("(j2 u t) (g s) -> j2 t u g s",
                                        j2=2, u=8, t=8, g=8)
                    tp_t = []
                    for t in range(8):
                        tpt = tpp.tile([128, SQ], BF16, tag=f"tp{t}")
                        for j2 in range(2):
                            nc.gpsimd.dma_start(
                                out=tpt[64 * j2: 64 * (j2 + 1), :],
                                in_=tg_v[j2, t])
                        tp_t.append(tpt)

                    # out conv + final selu
                    for t in range(8):
                        po = outps.tile([128, 1024], F32, tag="o")
                        nc.tensor.matmul(
                            po[:, 0:512], wo_sb, tp_t[t][:, 0:512],
                            start=True, stop=False)
                        nc.tensor.matmul(
                            po[:, 512:1024], wo_sb, tp_t[t][:, 512:1024],
                            start=True, stop=False)
                        nc.tensor.matmul(
                            po[:, 0:512], resI_sb,
                            x_sb[t][:, s0q: s0q + 512],
                            start=False, stop=True)
                        nc.tensor.matmul(
                            po[:, 512:1024], resI_sb,
                            x_sb[t][:, s0q + 512: s0q + 1024],
                            start=False, stop=True)
                        e2 = stg2.tile([128, 1024], F32, tag="e2")
                        nc.scalar.activation(
                            out=e2, in_=po,
                            func=mybir.ActivationFunctionType.Exp,
                            bias=be_sb[:, 0:1], scale=INV_LAM)
                        ot = outst.tile([128, 1024], BF16, tag="ot")
                        if not P2_B[comb_idx]:
                            nc.vector._custom_dve(
                                SELU_COMBINE, out=ot, in0=e2, in1=po,
                                s0=LA, s1=bra_sb[:, 0:1])
                        else:
                            rf = stg2.tile([128, 1024], BF16, tag="rf")
                            nc.scalar.activation(
                                out=rf, in_=po,
                                func=mybir.ActivationFunctionType.Relu,
                                bias=bra_sb[:, 0:1])
                            e2p = stg2.tile([128, 1024], BF16, tag="e2p")
                            nc.gpsimd.tensor_scalar(
                                out=e2p, in0=e2, scalar1=LA, scalar2=None,
                                op0=mybir.AluOpType.subtract)
                            nc.vector.tensor_tensor(
                                out=ot, in0=e2p, in1=rf,
                                op=mybir.AluOpType.min)
                        comb_idx += 1
                        nc.sync.dma_start(
                            out=out_d[t][:, s0q: s0q + 1024], in_=ot)
    nc.compile()
    return nc


_NC_CACHE = None


def _get_nc():
    global _NC_CACHE
    if _NC_CACHE is None:
        _NC_CACHE = build_nc()
    return _NC_CACHE


def kernel(in_tensor, w_value, w_key, w_query, w_out, b_out, **_ignored):
    in_tensor = np.asarray(in_tensor, dtype=np.float32)
    consts = host_constants(
        np.asarray(w_value, dtype=np.float32),
        np.asarray(w_key, dtype=np.float32),
        np.asarray(w_query, dtype=np.float32),
        np.asarray(w_out, dtype=np.float32),
        np.asarray(b_out, dtype=np.float32))
    assert in_tensor.shape[0] == 8
    in_maps = make_in_maps(in_tensor, consts)

    nc = _get_nc()
    from concourse.bass_utils import run_bass_kernel_spmd
    res = run_bass_kernel_spmd(nc, in_maps, core_ids=list(range(8)))
    outs = [np.asarray(res.results[b]["out"]).astype(np.float32)
            .reshape(K, C, 64, 64) for b in range(8)]
    return np.stack(outs, axis=0)


if __name__ == "__main__":
    build_nc()
    print("built ok")
